# revision 1
# baseline (speedup 1.0000x reference)
"""DissipativeThetaRINN Trainium2 (Bass/Tile) kernel — 8-core data parallel.

Strategy (pure data parallel, per sharding hint):
  - Batch B=2048 is split across 8 NeuronCores (256 rows/core); the tiny
    controller matrices and value-MLP weights are replicated.
  - On-device layout is transposed: features on SBUF partitions, batch on
    the free dimension.
  - Per timestep the implicit layer w = tanh(Cv x + Dvy y + Dvw w) is run
    as a fixed-point iteration. The batch is split into two 128-column
    chunks so chunk A's tanh (ScalarE) overlaps chunk B's matmuls (PE).
    The constant term is re-folded into PSUM by a second accumulating
    matmul each iteration, so ScalarE only does one Tanh per chunk.
  - The fixed point contracts with factor ~0.47/iter; N_ITERS iterations
    reproduce the reference's 30-iteration result to ~1e-4 (the
    reference's own iterate converges to fp32 noise by ~iteration 20).
  - Matmuls run in fp16 (PSUM accumulates fp32); the x recurrence keeps an
    fp32 accumulator on device, and DT is pre-folded into the recurrence
    weights so fp16 rounding only touches the 0.01-scaled increment.
  - The value MLP (independent of the recurrence) is computed in grouped
    timestep pairs and scheduled into the fixed-point loop's engine gaps.
  - log_stds broadcast and the +b2 value bias are applied host-side during
    output assembly.
"""
import numpy as np
import concourse.bass as bass
import concourse.mybir as mybir
import concourse.tile as tile
from concourse import bacc
from concourse.bass_utils import run_bass_kernel_spmd

dt = mybir.dt
AF = mybir.ActivationFunctionType

# problem shape (hardcoded per contract)
BFULL, TFULL = 2048, 128
S, NL, IN, OUT, H = 16, 128, 32, 8, 64
DT = 0.01
N_CORES = 8
N_ITERS = 11   # fixed-point tanh evaluations per timestep
VG = 2         # value-MLP timestep group


def build_kernel(T=TFULL, B=BFULL // N_CORES, n_iters=N_ITERS):
    nc = bacc.Bacc(None, target_bir_lowering=False)
    f32, f16 = dt.float32, dt.float16
    C = B // 2  # batch chunk width

    obsT16 = nc.dram_tensor("obsT16", [T, IN, B], f16, kind="ExternalInput")
    x0T = nc.dram_tensor("x0T", [S, B], f32, kind="ExternalInput")
    Wdvw = nc.dram_tensor("Wdvw", [NL, NL], f16, kind="ExternalInput")
    Wcd = nc.dram_tensor("Wcd", [S + IN, NL], f16, kind="ExternalInput")
    Wu = nc.dram_tensor("Wu", [S + IN, OUT], f16, kind="ExternalInput")
    Wuw = nc.dram_tensor("Wuw", [NL, OUT], f16, kind="ExternalInput")
    Wx = nc.dram_tensor("Wx", [S + IN, S], f16, kind="ExternalInput")
    Wxw = nc.dram_tensor("Wxw", [NL, S], f16, kind="ExternalInput")
    Wv0 = nc.dram_tensor("Wv0", [IN, H], f16, kind="ExternalInput")
    Wv1 = nc.dram_tensor("Wv1", [2 * H, H], f16, kind="ExternalInput")
    Wv2 = nc.dram_tensor("Wv2", [2 * H, 1], f16, kind="ExternalInput")
    b0v = nc.dram_tensor("b0v", [NL, 1], f32, kind="ExternalInput")
    b1v = nc.dram_tensor("b1v", [NL, 1], f32, kind="ExternalInput")

    u_out = nc.dram_tensor("u_out", [T, OUT, B], f32, kind="ExternalOutput")
    v_out = nc.dram_tensor("v_out", [T, B], f32, kind="ExternalOutput")

    NV = VG * B

    with tile.TileContext(nc) as tc:
        with tc.tile_pool(name="wts", bufs=1) as wts, \
             tc.tile_pool(name="xyp", bufs=3) as xyp, \
             tc.tile_pool(name="wp", bufs=2) as wp, \
             tc.tile_pool(name="iop", bufs=3) as iop, \
             tc.tile_pool(name="vp", bufs=2) as vp, \
             tc.tile_pool(name="pw0", bufs=2, space="PSUM") as pwp0, \
             tc.tile_pool(name="pw1", bufs=2, space="PSUM") as pwp1, \
             tc.tile_pool(name="pxp0", bufs=1, space="PSUM") as pxp0, \
             tc.tile_pool(name="pxp1", bufs=1, space="PSUM") as pxp1, \
             tc.tile_pool(name="pup", bufs=1, space="PSUM") as pup, \
             tc.tile_pool(name="phh", bufs=1, space="PSUM") as php:
            pwp = [pwp0, pwp1]

            def wt(name, dram, shape, dtp):
                tl = wts.tile(shape, dtp, name=name)
                nc.sync.dma_start(tl[:], dram[:])
                return tl
            wdvw = wt("wdvw", Wdvw, [NL, NL], f16)
            wcd = wt("wcd", Wcd, [S + IN, NL], f16)
            wu = wt("wu", Wu, [S + IN, OUT], f16)
            wuw = wt("wuw", Wuw, [NL, OUT], f16)
            wx = wt("wx", Wx, [S + IN, S], f16)
            wxw = wt("wxw", Wxw, [NL, S], f16)
            wv0 = wt("wv0", Wv0, [IN, H], f16)
            wv1 = wt("wv1", Wv1, [2 * H, H], f16)
            wv2 = wt("wv2", Wv2, [2 * H, 1], f16)
            b0 = wt("b0", b0v, [NL, 1], f32)
            b1 = wt("b1", b1v, [NL, 1], f32)

            # xy_h [48,B] f16: rows 0:32 = y^T, rows 32:48 = x^T; xt_r = fp32 x accum
            yst_h = iop.tile([IN, B], f16, name="yst_h0", tag="yst_h")
            nc.sync.dma_start(yst_h[:], obsT16[0])
            xt_r = xyp.tile([S, B], f32, name="xt_r0", tag="xt_r")
            nc.sync.dma_start(xt_r[:], x0T[:])
            xy_h = xyp.tile([S + IN, B], f16, name="xy_h0", tag="xy_h")
            nc.vector.tensor_copy(xy_h[0:IN, :], yst_h[:])
            nc.vector.tensor_copy(xy_h[IN:, :], xt_r[:])

            for t in range(T):
                # ---------- value MLP (grouped over VG timesteps) ----------
                if t % VG == 0:
                    with nc.named_scope(f"value_{t}"):
                        obs_v = vp.tile([IN, NV], f16, name=f"obs_v{t}", tag="obs_v")
                        osrc = obsT16[t:t + VG].transpose([1, 0, 2])
                        nc.sync.dma_start(obs_v[:].rearrange("k (g b) -> k g b", g=VG), osrc)
                        nvc = (NV + 511) // 512
                        ph = php.tile([H, NV], dt.float32, name=f"ph1_{t}", tag="ph")
                        for j in range(nvc):
                            js = slice(j * 512, min((j + 1) * 512, NV))
                            nc.tensor.matmul(ph[:, js], wv0[:], obs_v[:, js], start=True, stop=True)
                        h1 = vp.tile([H, NV], f16, name=f"h1_{t}", tag="h1")
                        nc.scalar.activation(h1[:], ph[:], AF.Tanh, bias=b0[0:H, :])
                        ph2 = php.tile([H, NV], dt.float32, name=f"ph2_{t}", tag="ph")
                        for j in range(nvc):
                            js = slice(j * 512, min((j + 1) * 512, NV))
                            nc.tensor.matmul(ph2[:, js], wv1[0:H, :], h1[:, js], start=True, stop=True)
                        h2 = vp.tile([H, NV], f16, name=f"h2_{t}", tag="h1")
                        nc.scalar.activation(h2[:], ph2[:], AF.Tanh, bias=b1[0:H, :])
                        v_sb = vp.tile([1, NV], f32, name=f"v_sb{t}", tag="v_sb")
                        for j in range(nvc):
                            js = slice(j * 512, min((j + 1) * 512, NV))
                            pv = php.tile([1, 512], dt.float32, name=f"pv{t}_{j}", tag="ph")
                            nc.tensor.matmul(pv[:, 0:js.stop - js.start], wv2[0:H, :], h2[:, js],
                                             start=True, stop=True)
                            nc.vector.tensor_copy(v_sb[:, js], pv[:, 0:js.stop - js.start])
                        nc.sync.dma_start(
                            v_out[t:t + VG].rearrange("g b -> (g b)").unsqueeze(0), v_sb[:])

                # ---------- fixed point, 2-chunk ping-pong ----------
                with nc.named_scope(f"fp_{t}"):
                    if t < T - 1:
                        # prefetch next y into the next xy tile
                        yst_h = iop.tile([IN, B], f16, name=f"ysth{t + 1}", tag="yst_h")
                        nc.sync.dma_start(yst_h[:], obsT16[t + 1])
                        xy_hn = xyp.tile([S + IN, B], f16, name=f"xyh{t + 1}", tag="xy_h")
                        nc.vector.tensor_copy(xy_hn[0:IN, :], yst_h[:])
                    w16 = [None, None]
                    for it in range(n_iters):
                        for c in range(2):
                            cs = slice(c * C, (c + 1) * C)
                            p = pwp[c].tile([NL, C], dt.float32, name=f"pw{t}_{it}_{c}", tag=f"pw{c}")
                            if it == 0:
                                nc.tensor.matmul(p[:], wcd[:], xy_h[:, cs], start=True, stop=True)
                            else:
                                nc.tensor.matmul(p[:], wcd[:], xy_h[:, cs], start=True, stop=False)
                                nc.tensor.matmul(p[:], wdvw[:], w16[c][:], start=False, stop=True)
                            wn = wp.tile([NL, C], f16, name=f"w{t}_{it}_{c}", tag=f"w{c}")
                            nc.scalar.activation(wn[:], p[:], AF.Tanh)
                            w16[c] = wn

                # ---------- x_next (critical path), then u ----------
                with nc.named_scope(f"out_{t}"):
                    if t < T - 1:
                        pxp = [pxp0, pxp1]
                        pxc = []
                        for c in range(2):
                            cs = slice(c * C, (c + 1) * C)
                            px = pxp[c].tile([S, C], dt.float32, name=f"px{t}_{c}", tag=f"px{c}")
                            nc.tensor.matmul(px[:], wx[:], xy_h[:, cs], start=True, stop=False)
                            nc.tensor.matmul(px[:], wxw[:], w16[c][:], start=False, stop=True)
                            # critical: fp16 x for the next step's const folds
                            nc.vector.tensor_add(xy_hn[IN:, cs], px[:], xt_r[:, cs])
                            pxc.append(px)
                        # off-critical: fp32 x accumulator
                        xt_rn = xyp.tile([S, B], f32, name=f"xtr{t + 1}", tag="xt_r")
                        for c in range(2):
                            cs = slice(c * C, (c + 1) * C)
                            nc.vector.tensor_add(xt_rn[:, cs], pxc[c][:], xt_r[:, cs])

                    pu = pup.tile([OUT, B], dt.float32, name=f"pu{t}", tag="pu")
                    nc.tensor.matmul(pu[:], wu[:], xy_h[:], start=True, stop=False)
                    for c in range(2):
                        cs = slice(c * C, (c + 1) * C)
                        nc.tensor.matmul(pu[:, cs], wuw[:], w16[c][:], start=False, stop=True)
                    u_sb = iop.tile([OUT, B], f32, name=f"u_sb{t}", tag="u_sb")
                    nc.vector.tensor_copy(u_sb[:], pu[:])
                    nc.sync.dma_start(u_out[t], u_sb[:])

                    if t < T - 1:
                        xt_r, xy_h = xt_rn, xy_hn

    nc.compile()
    return nc


def host_inputs(inputs, core, n_cores=N_CORES):
    BL = inputs["obs"].shape[0] // n_cores
    sl = slice(core * BL, (core + 1) * BL)
    obs = np.ascontiguousarray(np.asarray(inputs["obs"])[sl].transpose(1, 2, 0))
    x0T = np.ascontiguousarray(np.asarray(inputs["x0"])[sl].T)
    g = lambda k: np.asarray(inputs[k])
    return {
        "obsT16": obs.astype(np.float16),
        "x0T": x0T.astype(np.float32),
        "Wdvw": g("Dvw_T").astype(np.float16),
        "Wcd": np.concatenate([g("Dvy_T"), g("Cv_T")], 0).astype(np.float16),
        "Wu": np.concatenate([g("Duy_T"), g("Cu_T")], 0).astype(np.float16),
        "Wuw": g("Duw_T").astype(np.float16),
        "Wx": np.concatenate([DT * g("By_T"), DT * g("A_T")], 0).astype(np.float16),
        "Wxw": (DT * g("Bw_T")).astype(np.float16),
        "Wv0": g("W0").astype(np.float16),
        "Wv1": np.tile(g("W1"), (2, 1)).astype(np.float16),
        "Wv2": np.tile(g("W2"), (2, 1)).astype(np.float16),
        "b0v": np.tile(g("b0").reshape(H, 1), (2, 1)).astype(np.float32),
        "b1v": np.tile(g("b1").reshape(H, 1), (2, 1)).astype(np.float32),
    }


def assemble_output(results, inputs, n_cores=N_CORES):
    obs = np.asarray(inputs["obs"])
    Bfull, T = obs.shape[0], obs.shape[1]
    BL = Bfull // n_cores
    out = np.empty((Bfull, T, 2 * OUT + 1), np.float32)
    log_stds = np.asarray(inputs["log_stds"], np.float32)
    b2 = np.asarray(inputs["b2"], np.float32)
    for c in range(n_cores):
        sl = slice(c * BL, (c + 1) * BL)
        out[sl, :, :OUT] = results[c]["u_out"].transpose(2, 0, 1)
        out[sl, :, OUT:2 * OUT] = log_stds
        out[sl, :, 2 * OUT:] = results[c]["v_out"].T[:, :, None] + b2
    return out


_NC_CACHE = {}


def _get_nc(T):
    if T not in _NC_CACHE:
        _NC_CACHE[T] = build_kernel(T=T)
    return _NC_CACHE[T]


def run_on_hw(inputs, trace=False):
    """Run the SPMD kernel; returns (full_output, exec_time_ns_or_None)."""
    T = np.asarray(inputs["obs"]).shape[1]
    nc = _get_nc(T)
    in_maps = [host_inputs(inputs, c) for c in range(N_CORES)]
    last_err = None
    for attempt in range(3):
        try:
            res = run_bass_kernel_spmd(nc, in_maps, list(range(N_CORES)), trace=trace)
            return assemble_output(res.results, inputs), res.exec_time_ns
        except Exception as e:  # transient device failures: retry
            last_err = e
    raise last_err


def kernel(**inputs) -> np.ndarray:
    out, _ = run_on_hw(inputs, trace=False)
    return out

